# revision 1
# baseline (speedup 1.0000x reference)
"""Trainium2 Bass kernel for nn_BaseIODEModel (GNN message-passing ODE field).

Data-parallel over trajectories: z [81920, 4] is split across 8 NeuronCores
along dim 0 in multiples of B=10 (1024 trajectories / 10240 rows per core);
the small MLP weights are replicated. Edge gather/softplus/sum is local per
trajectory, so there is no cross-device communication.

Per-core program (all feature-major on chip, fp32 precision with float32r
matmuls on the PE):
  zT = transpose(z)                                 [4, cols]   (PE transpose)
  self-dynamics MLP:   softplus = ln(1 + exp(.)) via the ACT engine's
                       exp/ln table set (walrus has no native softplus set)
  interaction net: layer-0 factorizes over edges:
       pre(r,s) = a_r + b_s + ib0 with a = [iW0_p; iW0_vr].T z,
                                       b = [-iW0_p; iW0_vs].T z
       exp(pre) = exp(a + ib0) * exp(b)   -> exp on 10k node cols instead of
                                             92k edge cols; grid combine is a
                                             DVE multiply
       h0e = ln(1 + ea_r * eb_s)   (grid (t, d, r), s = (r+d) mod 10, d=1..9)
       h1e = ln(1 + exp(iW1.T h0e + ib1))
       dz_int = sum_d iW2.T h1e[:, :, d, :]    (PSUM accumulation)
  out = dz_self + dz_int + (fb2 + 9*ib2), PE-transposed back to row-major.
"""

import json
import os
import tempfile

import numpy as np



import concourse.bass as bass
import concourse.hw_specs as _hw_specs
import concourse.mybir as _mybir_for_tables
from concourse import bacc


def _patch_activation_tables():
    """Make Exp and Ln resolve to the combined natural_log_exp_and_others
    ACT table set. Bacc's insert_act_table_loads picks the first set that
    contains each function, which puts Exp and Ln in two different sets and
    inserts a ~1.3us ACT_TABLE_LOAD at every exp<->ln alternation (~160us
    across this kernel). Filtering the other sets' exp/ln entries keeps set
    ids stable (index into act_info.json) while forcing the shared set."""
    if getattr(_hw_specs, "_nle_patched", False):
        return
    orig = _hw_specs.get_activation_tables
    comb = "natural_log_exp_and_others"
    EXP = _mybir_for_tables.ActivationFunctionType.Exp
    LN = _mybir_for_tables.ActivationFunctionType.Ln

    def patched(module_arch):
        tables = orig(module_arch)
        if comb in tables and EXP in tables[comb] and LN in tables[comb]:
            for name, funcs in tables.items():
                if name != comb:
                    funcs.discard(EXP)
                    funcs.discard(LN)
        return tables

    _hw_specs.get_activation_tables = patched
    _hw_specs._nle_patched = True
    import concourse.bacc as _bacc_mod
    if getattr(_bacc_mod, "get_activation_tables", None) is orig:
        _bacc_mod.get_activation_tables = patched


_patch_activation_tables()
import concourse.mybir as mybir
import concourse.tile as tile
from concourse.bass_utils import run_bass_kernel_spmd
from concourse.masks import make_identity

F32 = mybir.dt.float32
F32R = mybir.dt.float32r
AF = mybir.ActivationFunctionType

B = 10          # objects per trajectory
NDIM = 2
NF = 2 * NDIM   # 4 features per node
H = 128         # hidden width (both MLPs)

N_CORES = 8
N_TRAJ = 8192           # total trajectories
N_LOC = N_TRAJ // N_CORES  # 1024 trajectories per core
ROWS = N_LOC * B        # 10240 node rows per core
GT = 128                # trajectories per group
NGROUP = N_LOC // GT    # 8 groups
GCOLS = GT * B          # 1280 node cols per group
TT = 32                 # trajectories per edge block
NBLK = GT // TT         # 4 edge blocks per group
BCOLS = TT * B          # 320 node cols per block
GRID = TT * (B - 1) * B  # 2880 grid cols per block

WEIGHT_NAMES = [
    "fW0", "fb0", "fW1", "fb1", "fW2", "fb2",
    "iW0", "ib0", "iW1", "ib1", "iW2", "ib2",
]


def _r(ap):
    return ap.bitcast(F32R)


def build(use_f32r=True, ngroup=NGROUP):
    mm = _r if use_f32r else (lambda x: x)
    rnd = mm  # producers of matmul inputs must write rounded-to-f32r values
    nc = bacc.Bacc()
    rows = ngroup * GCOLS

    z = nc.declare_dram_parameter("z", [rows, NF], F32, isOutput=False)
    w = {}
    for name, shp in [
        ("fW0", [NF, H]), ("fb0", [H]), ("fW1", [H, H]), ("fb1", [H]),
        ("fW2", [H, NF]), ("fb2", [NF]),
        ("iW0", [3 * NDIM, H]), ("ib0", [H]), ("iW1", [H, H]), ("ib1", [H]),
        ("iW2", [H, NF]), ("ib2", [NF]),
        ("Wb", [NF, H]), ("bias2", [NF]),
    ]:
        w[name] = nc.declare_dram_parameter(name, shp, F32, isOutput=False)
    out = nc.declare_dram_parameter("out", [rows, NF], F32, isOutput=True)

    # DRAM views: rows=(g,p,c): partition p = trajectory, c = node.
    # Per-partition runs are 10*4 contiguous f32 (160B DMA bursts).
    z_v = z.rearrange("(g p c) f -> g p (c f)", g=ngroup, p=128, c=B)
    out_v = out.rearrange("(g p c) f -> g p (c f)", g=ngroup, p=128, c=B)

    with tile.TileContext(nc) as tc:
        with (
            tc.tile_pool(name="const", bufs=1) as const,
            tc.tile_pool(name="zio", bufs=2) as zio,
            tc.tile_pool(name="nodes", bufs=2) as nodes,
            tc.tile_pool(name="grids", bufs=2) as grids,
            tc.tile_pool(name="outs", bufs=2) as outs,
            tc.tile_pool(name="misc_ps", bufs=1, space="PSUM") as misc_ps,
            tc.tile_pool(name="ab_ps", bufs=2, space="PSUM") as ab_ps,
            tc.tile_pool(name="edge_ps", bufs=2, space="PSUM") as edge_ps,
            tc.tile_pool(name="dz_ps", bufs=1, space="PSUM") as dz_ps,
        ):
            # ---- constants / weights ----
            ident128 = const.tile([128, 128], F32)
            make_identity(nc, ident128)
            ident4 = const.tile([NF, NF], F32)
            make_identity(nc, ident4)
            _zT0 = [None]

            def z_load(g):
                # ---- load z (contiguous) and transpose to feature-major ----
                z_sb = zio.tile([128, B, NF], F32)  # [traj, node, feat]
                nc.sync.dma_start(out=z_sb[:].rearrange("p c f -> p (c f)"),
                                  in_=z_v[g])

                # zT cols ordered (node r, traj t): col = r*128 + t
                zT_sb = zio.tile([NF, GCOLS], F32)
                for h in range(3):  # col chunks of 512,512,256
                    c0 = h * 512
                    c1 = min(GCOLS, c0 + 512)
                    zt_ps = misc_ps.tile([128, 512], F32, tag="misc")
                    for r in range(c0 // 128, c1 // 128):
                        nc.tensor.transpose(
                            zt_ps[0:NF, r * 128 - c0:(r + 1) * 128 - c0],
                            z_sb[:, r, :],
                            ident128[:],
                        )
                    nc.vector.tensor_copy(rnd(zT_sb[:, c0:c1]), zt_ps[0:NF, 0:c1 - c0])
                return zT_sb

            def node_phase(g, zT_sb=None):
                if zT_sb is None:
                    zT_sb = z_load(g)
                # ---- node terms: ea = exp(a+ib0), eb = exp(b) (dup x2) ----
                ea_sb = nodes.tile([H, B, GT], F32)          # (r, t)
                eb_ext = nodes.tile([H, 2 * B, GT], F32)     # (r mod 10, t)
                ea_f = ea_sb[:].rearrange("p r t -> p (r t)")
                eb_f = eb_ext[:].rearrange("p r t -> p (r t)")
                for h in range(3):
                    c0 = h * 512
                    c1 = min(GCOLS, c0 + 512)
                    wd = c1 - c0
                    a_ps = ab_ps.tile([128, 512], F32, tag="ab")
                    nc.tensor.matmul(
                        a_ps[:, 0:wd], mm(Wa_sb[:]), mm(zT_sb[:, c0:c1]))
                    nc.scalar.activation(
                        out=ea_f[:, c0:c1],
                        in_=a_ps[:, 0:wd], func=AF.Exp, bias=ib0_c[:], scale=1.0)
                    b_ps = ab_ps.tile([128, 512], F32, tag="ab")
                    nc.tensor.matmul(
                        b_ps[:, 0:wd], mm(Wb_sb[:]), mm(zT_sb[:, c0:c1]))
                    nc.scalar.activation(
                        out=eb_f[:, c0:c1],
                        in_=b_ps[:, 0:wd], func=AF.Exp, scale=1.0)
                # duplicate eb for cyclic sender indexing
                nc.vector.tensor_copy(eb_f[:, GCOLS:2 * GCOLS], eb_f[:, 0:GCOLS])

                # ---- self MLP (feature-major) ----
                h1s_sb = nodes.tile([H, GCOLS], F32)
                for h in range(3):
                    c0 = h * 512
                    c1 = min(GCOLS, c0 + 512)
                    wd = c1 - c0
                    s0_ps = ab_ps.tile([128, 512], F32, tag="ab")
                    nc.tensor.matmul(s0_ps[:, 0:wd], mm(fW0_sb[:]), mm(zT_sb[:, c0:c1]))
                    t0s = zio.tile([H, 512], F32, tag="t0s")
                    nc.scalar.activation(out=t0s[:, 0:wd], in_=s0_ps[:, 0:wd],
                                         func=AF.Exp, bias=fb0_c[:], scale=1.0)
                    h0s = zio.tile([H, 512], F32, tag="h0s")
                    nc.scalar.activation(out=rnd(h0s[:, 0:wd]), in_=t0s[:, 0:wd],
                                         func=AF.Ln, bias=1.0, scale=1.0)
                    s1_ps = ab_ps.tile([128, 512], F32, tag="ab")
                    nc.tensor.matmul(s1_ps[:, 0:wd], mm(fW1_sb[:]), mm(h0s[:, 0:wd]))
                    t1s = zio.tile([H, 512], F32, tag="t1s")
                    nc.scalar.activation(out=t1s[:, 0:wd], in_=s1_ps[:, 0:wd],
                                         func=AF.Exp, bias=fb1_c[:], scale=1.0)
                    nc.scalar.activation(out=rnd(h1s_sb[:, c0:c1]), in_=t1s[:, 0:wd],
                                         func=AF.Ln, bias=1.0, scale=1.0)
                return ea_sb, eb_ext, h1s_sb

            def edge_phase(g, ea_sb, eb_ext, h1s_sb, prefetch_g=None):
                h1s_v = h1s_sb[:].rearrange("p (r t) -> p r t", r=B)
                out_sb = outs.tile([NF, B, GT], F32)  # (r, t)
                nxt = None

                def grid_ln(k):
                    # DVE grid combine + ACT ln for block k; emitted one
                    # block ahead so ACT has this while PE runs L1 matmuls.
                    tsl = slice(k * TT, (k + 1) * TT)
                    t0 = grids.tile([H, B - 1, B, TT], F32)  # (d, r, t)
                    for d in range(1, B):
                        nc.vector.tensor_mul(
                            t0[:, d - 1, :, :],
                            ea_sb[:, :, tsl],
                            eb_ext[:, d:d + B, tsl],
                        )
                    g0 = grids.tile([H, GRID], F32)
                    nc.scalar.activation(
                        out=rnd(g0[:]), in_=t0[:], func=AF.Ln, bias=1.0, scale=1.0)
                    return g0

                g0 = grid_ln(0)
                for k in range(NBLK):
                    if prefetch_g is not None and k == 1:
                        nxt = node_phase(prefetch_g)
                    tsl = slice(k * TT, (k + 1) * TT)
                    g0_next = grid_ln(k + 1) if k + 1 < NBLK else None
                    t1 = grids.tile([H, B - 1, B, TT], F32)
                    t1f = t1[:].rearrange("p d r t -> p (d r t)")
                    for third in range(3):
                        e_ps = edge_ps.tile([128, 960], F32)
                        base = third * 960
                        for q0, q1 in [(0, 512), (512, 960)]:
                            nc.tensor.matmul(
                                e_ps[:, q0:q1],
                                mm(iW1_sb[:]),
                                mm(g0[:, base + q0:base + q1]))
                        nc.scalar.activation(
                            out=t1f[:, base:base + 960],
                            in_=e_ps[:], func=AF.Exp, bias=ib1_c[:], scale=1.0)
                    h1e = grids.tile([H, B - 1, B, TT], F32)
                    nc.scalar.activation(
                        out=rnd(h1e[:]), in_=t1[:], func=AF.Ln, bias=1.0, scale=1.0)

                    dzp = dz_ps.tile([NF, BCOLS], F32)  # cols (r, t-block)
                    nc.tensor.matmul(dzp[:], mm(fW2_sb[:]),
                                     mm(h1s_v[:, :, tsl]),
                                     start=True, stop=False)
                    for d in range(1, B):
                        nc.tensor.matmul(dzp[:], mm(iW2_sb[:]),
                                         mm(h1e[:, d - 1, :, :]),
                                         start=False, stop=(d == B - 1))
                    nc.vector.tensor_scalar_add(
                        out_sb[:, :, tsl], dzp[:], bias2[:])
                    g0 = g0_next

                # ---- transpose back and store (contiguous) ----
                ot_ps = misc_ps.tile([128, 512], F32, tag="misc")
                out_f = out_sb[:].rearrange("p r t -> p (r t)")
                for r in range(B):
                    nc.tensor.transpose(
                        ot_ps[:, r * NF:(r + 1) * NF],
                        out_f[:, r * 128:(r + 1) * 128],
                        ident4[:],
                    )
                outT_sb = outs.tile([128, B, NF], F32)
                nc.vector.tensor_copy(outT_sb[:], ot_ps[:, 0:B * NF])
                nc.sync.dma_start(out=out_v[g],
                                  in_=outT_sb[:].rearrange("p c f -> p (c f)"))
                return nxt

            _zT0[0] = z_load(0)

            def weight_tile(p, fdim, name, src_ap):
                # DMA to staging, then round on DVE so the tile qualifies as
                # an FP32R matmul input (walrus checkMatmultFP32r).
                stage = const.tile([p, fdim], F32, tag=f"wstage_{name}")
                nc.sync.dma_start(out=stage[:], in_=src_ap)
                t = const.tile([p, fdim], F32, tag=f"w_{name}")
                nc.vector.tensor_copy(rnd(t[:]), stage[:])
                return t

            fW0_sb = weight_tile(NF, H, "fW0", w["fW0"][:])
            fW1_sb = weight_tile(H, H, "fW1", w["fW1"][:])
            fW2_sb = weight_tile(H, NF, "fW2", w["fW2"][:])
            iW1_sb = weight_tile(H, H, "iW1", w["iW1"][:])
            iW2_sb = weight_tile(H, NF, "iW2", w["iW2"][:])

            # Wa = iW0[0:4]  (pos-part rows 0:2, v_recv rows 2:4)
            Wa_sb = weight_tile(NF, H, "Wa", w["iW0"][0:NF, :])
            # Wb = [-iW0[0:2]; iW0[4:6]] is prepared host-side (param "Wb")
            Wb_sb = weight_tile(NF, H, "Wb", w["Wb"][:])

            # bias columns [P,1]
            def bias_col(p, name):
                t = const.tile([p, 1], F32, tag=f"bias_{name}")
                nc.sync.dma_start(out=t[:], in_=w[name].rearrange("(a b) -> a b", b=1))
                return t

            fb0_c = bias_col(H, "fb0")
            fb1_c = bias_col(H, "fb1")
            ib0_c = bias_col(H, "ib0")
            ib1_c = bias_col(H, "ib1")
            # bias2 = fb2 + 9*ib2 is prepared host-side (param "bias2")
            bias2 = bias_col(NF, "bias2")

            # software-pipelined: group g+1's node phase is emitted after
            # group g's first edge block, so its PE/DVE prefetch work runs
            # while ACT chews on g's grid, without blocking g's ACT queue.
            tiles = node_phase(0, _zT0[0])
            for g in range(ngroup):
                pf = g + 1 if g + 1 < ngroup else None
                tiles = edge_phase(g, *tiles, prefetch_g=pf)

    nc.finalize()
    return nc


_NC_CACHE = {}


def _get_nc():
    if "nc" not in _NC_CACHE:
        _NC_CACHE["nc"] = build()
    return _NC_CACHE["nc"]


def run(inputs, trace=False, **kwargs):
    """Shard, run on 8 cores, gather. Returns (out, BassKernelResults)."""
    nc = _get_nc()
    z = np.ascontiguousarray(np.asarray(inputs["z"], dtype=np.float32))
    assert z.shape == (N_TRAJ * B, NF), z.shape
    weights = {k: np.ascontiguousarray(np.asarray(inputs[k], dtype=np.float32))
               for k in WEIGHT_NAMES}
    iW0 = weights["iW0"]
    weights["Wb"] = np.ascontiguousarray(
        np.concatenate([-iW0[0:NDIM], iW0[2 * NDIM:3 * NDIM]], axis=0))
    weights["bias2"] = np.ascontiguousarray(
        weights["fb2"] + (B - 1) * weights["ib2"])
    in_maps = []
    for c in range(N_CORES):
        m = dict(weights)
        m["z"] = z[c * ROWS:(c + 1) * ROWS]
        in_maps.append(m)
    res = run_bass_kernel_spmd(nc, in_maps, list(range(N_CORES)),
                               trace=trace, **kwargs)
    out = np.concatenate([res.results[c]["out"] for c in range(N_CORES)], axis=0)
    return out, res


def kernel(**inputs) -> np.ndarray:
    out, _ = run(inputs)
    return out



# revision 3
# speedup vs baseline: 1.1491x; 1.1491x over previous
"""Trainium2 Bass kernel for nn_BaseIODEModel (GNN message-passing ODE field).

Data-parallel over trajectories: z [81920, 4] is split across 8 NeuronCores
along dim 0 (1024 trajectories / 10240 rows per core); the small MLP weights
are replicated. Edge gather/softplus/sum is local per trajectory, so there is
no cross-device communication.

Per-core program, fp16 matmul datapath (PE runs 2-byte dtypes at 1 col/cycle
vs 2-4 cycles for fp32r; fp32 PSUM accumulation throughout; ranges verified:
max |a_r + b_s| < 9.2 so exp products stay < 1e4, well inside fp16):

  zT = transpose(z)                          [4, 1280] per group (PE)
  interaction layer 0 factorizes over edges:
       pre(r,s) = a_r + b_s + ib0,  a = [iW0_p; iW0_vr].T z,
                                    b = [-iW0_p; iW0_vs].T z
       ea = exp(a + ib0), eb = exp(b)   (ACT, node columns only)
       t0(d,r,t) = ea_r * eb_{(r+d)%B}  (DVE fp16 2x, one op per shift d)
       g0 = ln(1 + t0)                  (ACT, one 2880-col op per block)
  layer 1:  u = exp(iW1.T g0 + ib1)     (ACT, PSUM in, f32 out)
            h1e = ln(1 + u)             (ACT, one 2880-col op per block)
  dz_int = sum_d iW2.T h1e[:, d]        (PE, PSUM accumulation)
  self-dynamics MLP: same exp/ln softplus pairs on node columns.
  out = dz_self + dz_int + (fb2 + 9*ib2), PE-transposed back to row-major.

ACT (the only transcendental engine; throughput is dtype-independent at
~0.86 ns/col) is the roofline: 3 edge passes x 92160 cols + 6 node passes
x 10240 cols per core. All ACT instructions are sized 1280-2880 cols to
amortize the ~200 ns per-instruction access latency.
"""

import numpy as np

import concourse.bass as bass
import concourse.hw_specs as _hw_specs
import concourse.mybir as _mybir_for_tables
from concourse import bacc


def _patch_activation_tables():
    """Make Exp and Ln resolve to the combined natural_log_exp_and_others
    ACT table set. Bacc's insert_act_table_loads picks the first set that
    contains each function, which puts Exp and Ln in two different sets and
    inserts a ~1.3us ACT_TABLE_LOAD at every exp<->ln alternation. Filtering
    the other sets' exp/ln entries keeps set ids stable (index into
    act_info.json) while forcing the shared set."""
    if getattr(_hw_specs, "_nle_patched", False):
        return
    orig = _hw_specs.get_activation_tables
    comb = "natural_log_exp_and_others"
    EXP = _mybir_for_tables.ActivationFunctionType.Exp
    LN = _mybir_for_tables.ActivationFunctionType.Ln

    def patched(module_arch):
        tables = orig(module_arch)
        if comb in tables and EXP in tables[comb] and LN in tables[comb]:
            for name, funcs in tables.items():
                if name != comb:
                    funcs.discard(EXP)
                    funcs.discard(LN)
        return tables

    _hw_specs.get_activation_tables = patched
    _hw_specs._nle_patched = True
    import concourse.bacc as _bacc_mod
    if getattr(_bacc_mod, "get_activation_tables", None) is orig:
        _bacc_mod.get_activation_tables = patched


_patch_activation_tables()
import concourse.mybir as mybir
import concourse.tile as tile
from concourse.bass_utils import run_bass_kernel_spmd
from concourse.masks import make_identity

F32 = mybir.dt.float32
F16 = mybir.dt.float16
AF = mybir.ActivationFunctionType

B = 10          # objects per trajectory
NDIM = 2
NF = 2 * NDIM   # 4 features per node
H = 128         # hidden width (both MLPs)

N_CORES = 8
N_TRAJ = 8192             # total trajectories
N_LOC = N_TRAJ // N_CORES  # 1024 trajectories per core
ROWS = N_LOC * B          # 10240 node rows per core
GT = 128                  # trajectories per group
NGROUP = N_LOC // GT      # 8 groups
GCOLS = GT * B            # 1280 node cols per group
TT = 32                   # trajectories per edge block
NBLK = GT // TT           # 4 edge blocks per group
GRID = TT * (B - 1) * B   # 2880 grid cols per block

WEIGHT_NAMES = [
    "fW0", "fb0", "fW1", "fb1", "fW2", "fb2",
    "iW0", "ib0", "iW1", "ib1", "iW2", "ib2",
]


def build(ngroup=NGROUP):
    nc = bacc.Bacc()
    rows = ngroup * GCOLS

    z = nc.declare_dram_parameter("z", [rows, NF], F32, isOutput=False)
    w = {}
    for name, shp in [
        ("fW0", [NF, H]), ("fb0", [H]), ("fW1", [H, H]), ("fb1", [H]),
        ("fW2", [H, NF]),
        ("iW0", [3 * NDIM, H]), ("ib0", [H]), ("iW1", [H, H]), ("ib1", [H]),
        ("iW2", [H, NF]),
        ("Wb", [NF, H]), ("bias2", [NF]),
    ]:
        w[name] = nc.declare_dram_parameter(name, shp, F32, isOutput=False)
    out = nc.declare_dram_parameter("out", [rows, NF], F32, isOutput=True)

    # DRAM views: rows=(g,p,c): partition p = trajectory, c = node.
    # Per-partition runs are 10*4 contiguous f32 (160B DMA bursts).
    z_v = z.rearrange("(g p c) f -> g p (c f)", g=ngroup, p=128, c=B)
    out_v = out.rearrange("(g p c) f -> g p (c f)", g=ngroup, p=128, c=B)

    with tile.TileContext(nc) as tc:
        with (
            tc.tile_pool(name="const", bufs=1) as const,
            tc.tile_pool(name="zio", bufs=2) as zio,
            tc.tile_pool(name="nodes", bufs=2) as nodes,
            tc.tile_pool(name="grids", bufs=2) as grids,
            tc.tile_pool(name="outs", bufs=2) as outs,
            # One shared PSUM ring for all [128, x] matmul outputs:
            # 1536 f32 = 3 banks; x2 bufs = 6 banks.  dz: [4,320] x2 = 2.
            tc.tile_pool(name="ab_ps", bufs=2, space="PSUM") as ab_ps,
            tc.tile_pool(name="dz_ps", bufs=2, space="PSUM") as dz_ps,
        ):
            ident128 = const.tile([128, 128], F16)
            make_identity(nc, ident128)
            ident4 = const.tile([NF, NF], F32)
            make_identity(nc, ident4)

            def ps_take():
                ab_take = ab_ps.tile([128, 1536], F32, tag="ab")
                return ab_take

            def z_load(g):
                # load z (contiguous), cast to fp16, transpose to [4, 1280]
                z_sb = zio.tile([128, B, NF], F32)  # [traj, node, feat]
                nc.sync.dma_start(out=z_sb[:].rearrange("p c f -> p (c f)"),
                                  in_=z_v[g])
                zb = zio.tile([128, B, NF], F16)
                nc.vector.tensor_copy(zb[:], z_sb[:])
                zt_ps = ps_take()
                ztv = zt_ps[:].bitcast(F16)  # [128, 3072] fp16 view
                for r in range(B):
                    nc.tensor.transpose(
                        ztv[0:NF, r * 128:(r + 1) * 128], zb[:, r, :],
                        ident128[:])
                zT = nodes.tile([NF, GCOLS], F16)
                nc.vector.tensor_copy(zT[:], ztv[0:NF, 0:GCOLS])
                return zT

            def mm_node(w_sb, rhs, act_func, act_bias, out_ap):
                # W.T @ rhs over GCOLS in bank-aligned chunks, then one
                # whole-width ACT pass PSUM -> SBUF.
                ps = ps_take()
                for c0, c1 in [(0, 512), (512, 1024), (1024, GCOLS)]:
                    nc.tensor.matmul(ps[:, c0:c1], w_sb[:], rhs[:, c0:c1],
                                     start=True, stop=True)
                nc.scalar.activation(out=out_ap, in_=ps[:, 0:GCOLS],
                                     func=act_func, bias=act_bias, scale=1.0)

            def node_phase(g, zT=None):
                if zT is None:
                    zT = z_load(g)
                # edge layer-0 node terms: ea = exp(a+ib0), eb = exp(b) (x2)
                ea = nodes.tile([H, B, GT], F16)
                eb_ext = nodes.tile([H, 2 * B, GT], F16)
                ea_f = ea[:].rearrange("p r t -> p (r t)")
                eb_f = eb_ext[:].rearrange("p r t -> p (r t)")
                mm_node(Wa_sb, zT, AF.Exp, ib0_c[:], ea_f[:, 0:GCOLS])
                mm_node(Wb_sb, zT, AF.Exp, 0.0, eb_f[:, 0:GCOLS])
                nc.vector.tensor_copy(eb_f[:, GCOLS:2 * GCOLS], eb_f[:, 0:GCOLS])

                # self-dynamics MLP (softplus = exp pass + ln pass)
                u0 = nodes.tile([H, GCOLS], F32, tag="u0")
                mm_node(fW0_sb, zT, AF.Exp, fb0_c[:], u0[:])
                h0s = nodes.tile([H, GCOLS], F16, tag="h0s")
                nc.scalar.activation(out=h0s[:], in_=u0[:], func=AF.Ln,
                                     bias=1.0, scale=1.0)
                u1 = nodes.tile([H, GCOLS], F32, tag="u1")
                mm_node(fW1_sb, h0s, AF.Exp, fb1_c[:], u1[:])
                h1s = nodes.tile([H, B, GT], F16, tag="h1s")
                nc.scalar.activation(
                    out=h1s[:].rearrange("p r t -> p (r t)"), in_=u1[:],
                    func=AF.Ln, bias=1.0, scale=1.0)
                return ea, eb_ext, h1s

            def edge_phase(g, ea, eb_ext, h1s, prefetch_g=None):
                nxt = None
                out_sb = outs.tile([NF, B, GT], F32)

                # grid combine for the whole group: t0(d,r,t) = ea_r*eb_{r+d}
                t0g = grids.tile([H, B - 1, B, GT], F16, tag="t0g")
                for d in range(1, B):
                    nc.vector.tensor_mul(
                        t0g[:, d - 1, :, :], ea[:], eb_ext[:, d:d + B, :])

                def grid_ln(k):
                    tsl = slice(k * TT, (k + 1) * TT)
                    g0 = grids.tile([H, GRID], F16, tag="g0")
                    nc.scalar.activation(
                        out=g0[:], in_=t0g[:, :, :, tsl], func=AF.Ln,
                        bias=1.0, scale=1.0)
                    return g0

                g0 = grid_ln(0)
                for k in range(NBLK):
                    if prefetch_g is not None and k == 1:
                        nxt = node_phase(prefetch_g)
                    tsl = slice(k * TT, (k + 1) * TT)
                    g0_next = grid_ln(k + 1) if k + 1 < NBLK else None

                    # layer 1: u = exp(iW1.T g0 + ib1) in two halves
                    u_e = grids.tile([H, GRID], F32, tag="u_e")
                    for base, width in [(0, 1536), (1536, GRID - 1536)]:
                        ps = ps_take()
                        for c0 in range(0, width, 512):
                            c1 = min(width, c0 + 512)
                            nc.tensor.matmul(
                                ps[:, c0:c1], iW1_sb[:],
                                g0[:, base + c0:base + c1],
                                start=True, stop=True)
                        nc.scalar.activation(
                            out=u_e[:, base:base + width], in_=ps[:, 0:width],
                            func=AF.Exp, bias=ib1_c[:], scale=1.0)
                    h1e = grids.tile([H, B - 1, B, TT], F16, tag="h1e")
                    nc.scalar.activation(
                        out=h1e[:].rearrange("p d r t -> p (d r t)"),
                        in_=u_e[:], func=AF.Ln, bias=1.0, scale=1.0)

                    # dz = fW2.T h1s + sum_d iW2.T h1e[:, d]
                    dzp = dz_ps.tile([NF, B * TT], F32)
                    nc.tensor.matmul(dzp[:], fW2_sb[:],
                                     h1s[:, :, tsl], start=True, stop=False)
                    for d in range(1, B):
                        nc.tensor.matmul(dzp[:], iW2_sb[:],
                                         h1e[:, d - 1, :, :],
                                         start=False, stop=(d == B - 1))
                    nc.vector.tensor_scalar_add(
                        out_sb[:, :, tsl], dzp[:], bias2_c[:])
                    g0 = g0_next

                # transpose back and store (contiguous)
                ot_ps = ps_take()
                out_f = out_sb[:].rearrange("p r t -> p (r t)")
                for r in range(B):
                    nc.tensor.transpose(
                        ot_ps[:, r * NF:(r + 1) * NF],
                        out_f[:, r * 128:(r + 1) * 128], ident4[:])
                outT = zio.tile([128, B, NF], F32, tag="outT")
                nc.vector.tensor_copy(outT[:], ot_ps[:, 0:B * NF])
                nc.sync.dma_start(out=out_v[g],
                                  in_=outT[:].rearrange("p c f -> p (c f)"))
                return nxt

            zT0 = z_load(0)

            def weight_tile(p, fdim, name, src_ap):
                stage = const.tile([p, fdim], F32, tag=f"wstage_{name}")
                nc.sync.dma_start(out=stage[:], in_=src_ap)
                t = const.tile([p, fdim], F16, tag=f"w_{name}")
                nc.vector.tensor_copy(t[:], stage[:])
                return t

            fW0_sb = weight_tile(NF, H, "fW0", w["fW0"][:])
            fW1_sb = weight_tile(H, H, "fW1", w["fW1"][:])
            fW2_sb = weight_tile(H, NF, "fW2", w["fW2"][:])
            iW1_sb = weight_tile(H, H, "iW1", w["iW1"][:])
            iW2_sb = weight_tile(H, NF, "iW2", w["iW2"][:])
            Wa_sb = weight_tile(NF, H, "Wa", w["iW0"][0:NF, :])
            # Wb = [-iW0[0:2]; iW0[4:6]] prepared host-side (param "Wb")
            Wb_sb = weight_tile(NF, H, "Wb", w["Wb"][:])

            def bias_col(p, name):
                t = const.tile([p, 1], F32, tag=f"bias_{name}")
                nc.sync.dma_start(out=t[:],
                                  in_=w[name].rearrange("(a b) -> a b", b=1))
                return t

            fb0_c = bias_col(H, "fb0")
            fb1_c = bias_col(H, "fb1")
            ib0_c = bias_col(H, "ib0")
            ib1_c = bias_col(H, "ib1")
            # bias2 = fb2 + 9*ib2 prepared host-side (param "bias2")
            bias2_c = bias_col(NF, "bias2")

            # software-pipelined: group g+1's node phase is emitted after
            # group g's first edge block so ACT never starves.
            tiles = node_phase(0, zT0)
            for g in range(ngroup):
                pf = g + 1 if g + 1 < ngroup else None
                tiles = edge_phase(g, *tiles, prefetch_g=pf)

    nc.finalize()
    return nc


_NC_CACHE = {}


def _get_nc():
    if "nc" not in _NC_CACHE:
        _NC_CACHE["nc"] = build()
    return _NC_CACHE["nc"]


def run(inputs, trace=False, **kwargs):
    """Shard, run on 8 cores, gather. Returns (out, BassKernelResults)."""
    nc = _get_nc()
    z = np.ascontiguousarray(np.asarray(inputs["z"], dtype=np.float32))
    assert z.shape == (N_TRAJ * B, NF), z.shape
    weights = {k: np.ascontiguousarray(np.asarray(inputs[k], dtype=np.float32))
               for k in WEIGHT_NAMES}
    iW0 = weights["iW0"]
    weights["Wb"] = np.ascontiguousarray(
        np.concatenate([-iW0[0:NDIM], iW0[2 * NDIM:3 * NDIM]], axis=0))
    weights["bias2"] = np.ascontiguousarray(
        weights["fb2"] + (B - 1) * weights["ib2"])
    for name in ("fb2", "ib2"):
        del weights[name]
    in_maps = []
    for c in range(N_CORES):
        m = dict(weights)
        m["z"] = z[c * ROWS:(c + 1) * ROWS]
        in_maps.append(m)
    res = run_bass_kernel_spmd(nc, in_maps, list(range(N_CORES)),
                               trace=trace, **kwargs)
    out = np.concatenate([res.results[c]["out"] for c in range(N_CORES)],
                         axis=0)
    return out, res


def kernel(**inputs) -> np.ndarray:
    out, _ = run(inputs)
    return out


# revision 5
# speedup vs baseline: 1.1610x; 1.0104x over previous
"""Trainium2 Bass kernel for nn_BaseIODEModel (GNN message-passing ODE field).

Data-parallel over trajectories: z [81920, 4] is split across 8 NeuronCores
along dim 0 (1024 trajectories / 10240 rows per core); the small MLP weights
are replicated. Edge gather/softplus/sum is local per trajectory, so there is
no cross-device communication.

Per-core program, fp16 matmul datapath (PE runs 2-byte dtypes at 1 col/cycle
vs 2-4 cycles for fp32r; fp32 PSUM accumulation throughout; ranges verified:
max |a_r + b_s| < 9.2 so exp products stay < 1e4, well inside fp16):

  zT = transpose(z)                          [4, 1280] per group (PE)
  interaction layer 0 factorizes over edges:
       pre(r,s) = a_r + b_s + ib0,  a = [iW0_p; iW0_vr].T z,
                                    b = [-iW0_p; iW0_vs].T z
       ea = exp(a + ib0), eb = exp(b)   (ACT, node columns only)
       t0(d,r,t) = ea_r * eb_{(r+d)%B}  (DVE fp16 2x, one op per shift d)
       g0 = ln(1 + t0)                  (ACT, one 2880-col op per block)
  layer 1:  u = exp(iW1.T g0 + ib1)     (ACT, PSUM in, f32 out)
            h1e = ln(1 + u)             (ACT, one 2880-col op per block)
  dz_int = sum_d iW2.T h1e[:, d]        (PE, PSUM accumulation)
  self-dynamics MLP: same exp/ln softplus pairs on node columns.
  out = dz_self + dz_int + (fb2 + 9*ib2), PE-transposed back to row-major.

ACT (the only transcendental engine; throughput is dtype-independent at
~0.86 ns/col) is the roofline: 3 edge passes x 92160 cols + 6 node passes
x 10240 cols per core. All ACT instructions are sized 1280-2880 cols to
amortize the ~200 ns per-instruction access latency.
"""

import numpy as np

import concourse.bass as bass
import concourse.hw_specs as _hw_specs
import concourse.mybir as _mybir_for_tables
from concourse import bacc


def _patch_activation_tables():
    """Make Exp and Ln resolve to the combined natural_log_exp_and_others
    ACT table set. Bacc's insert_act_table_loads picks the first set that
    contains each function, which puts Exp and Ln in two different sets and
    inserts a ~1.3us ACT_TABLE_LOAD at every exp<->ln alternation. Filtering
    the other sets' exp/ln entries keeps set ids stable (index into
    act_info.json) while forcing the shared set."""
    if getattr(_hw_specs, "_nle_patched", False):
        return
    orig = _hw_specs.get_activation_tables
    comb = "natural_log_exp_and_others"
    EXP = _mybir_for_tables.ActivationFunctionType.Exp
    LN = _mybir_for_tables.ActivationFunctionType.Ln

    def patched(module_arch):
        tables = orig(module_arch)
        if comb in tables and EXP in tables[comb] and LN in tables[comb]:
            for name, funcs in tables.items():
                if name != comb:
                    funcs.discard(EXP)
                    funcs.discard(LN)
        return tables

    _hw_specs.get_activation_tables = patched
    _hw_specs._nle_patched = True
    import concourse.bacc as _bacc_mod
    if getattr(_bacc_mod, "get_activation_tables", None) is orig:
        _bacc_mod.get_activation_tables = patched


_patch_activation_tables()
import concourse.mybir as mybir
import concourse.tile as tile
from concourse.bass_utils import run_bass_kernel_spmd
from concourse.masks import make_identity

F32 = mybir.dt.float32
F16 = mybir.dt.float16
AF = mybir.ActivationFunctionType

B = 10          # objects per trajectory
NDIM = 2
NF = 2 * NDIM   # 4 features per node
H = 128         # hidden width (both MLPs)

N_CORES = 8
N_TRAJ = 8192             # total trajectories
N_LOC = N_TRAJ // N_CORES  # 1024 trajectories per core
ROWS = N_LOC * B          # 10240 node rows per core
GT = 128                  # trajectories per group
NGROUP = N_LOC // GT      # 8 groups
GCOLS = GT * B            # 1280 node cols per group
TT = 32                   # trajectories per edge block
NBLK = GT // TT           # 4 edge blocks per group
GRID = TT * (B - 1) * B   # 2880 grid cols per block

WEIGHT_NAMES = [
    "fW0", "fb0", "fW1", "fb1", "fW2", "fb2",
    "iW0", "ib0", "iW1", "ib1", "iW2", "ib2",
]


def build(ngroup=NGROUP):
    nc = bacc.Bacc()
    rows = ngroup * GCOLS

    z = nc.declare_dram_parameter("z", [rows, NF], F32, isOutput=False)
    w = {}
    for name, shp in [
        ("fW0", [NF, H]), ("fb0", [H]), ("fW1", [H, H]), ("fb1", [H]),
        ("fW2", [H, NF]),
        ("iW0", [3 * NDIM, H]), ("ib0", [H]), ("iW1", [H, H]), ("ib1", [H]),
        ("iW2", [H, NF]),
        ("Wb", [NF, H]), ("bias2", [NF]),
    ]:
        w[name] = nc.declare_dram_parameter(name, shp, F32, isOutput=False)
    out = nc.declare_dram_parameter("out", [rows, NF], F32, isOutput=True)

    # DRAM views: rows=(g,p,c): partition p = trajectory, c = node.
    # Per-partition runs are 10*4 contiguous f32 (160B DMA bursts).
    z_v = z.rearrange("(g p c) f -> g p (c f)", g=ngroup, p=128, c=B)
    out_v = out.rearrange("(g p c) f -> g p (c f)", g=ngroup, p=128, c=B)

    with tile.TileContext(nc) as tc:
        with (
            tc.tile_pool(name="const", bufs=1) as const,
            tc.tile_pool(name="zio", bufs=2) as zio,
            tc.tile_pool(name="nodes", bufs=2) as nodes,
            tc.tile_pool(name="grids", bufs=2) as grids,
            tc.tile_pool(name="outs", bufs=2) as outs,
            # One shared PSUM ring for all [128, x] matmul outputs:
            # 1536 f32 = 3 banks; x2 bufs = 6 banks.  dz: [4,320] x2 = 2.
            tc.tile_pool(name="ab_ps", bufs=2, space="PSUM") as ab_ps,
            tc.tile_pool(name="dz_ps", bufs=2, space="PSUM") as dz_ps,
        ):
            ident128 = const.tile([128, 128], F16)
            make_identity(nc, ident128)
            ident4 = const.tile([NF, NF], F32)
            make_identity(nc, ident4)

            def ps_take():
                ab_take = ab_ps.tile([128, 1536], F32, tag="ab")
                return ab_take

            def z_load(g):
                # load z (contiguous), cast to fp16, transpose to [4, 1280]
                z_sb = zio.tile([128, B, NF], F32)  # [traj, node, feat]
                nc.sync.dma_start(out=z_sb[:].rearrange("p c f -> p (c f)"),
                                  in_=z_v[g])
                zb = zio.tile([128, B, NF], F16)
                nc.vector.tensor_copy(zb[:], z_sb[:])
                zt_ps = ps_take()
                ztv = zt_ps[:].bitcast(F16)  # [128, 3072] fp16 view
                for r in range(B):
                    nc.tensor.transpose(
                        ztv[0:NF, r * 128:(r + 1) * 128], zb[:, r, :],
                        ident128[:])
                zT = nodes.tile([NF, GCOLS], F16)
                nc.vector.tensor_copy(zT[:], ztv[0:NF, 0:GCOLS])
                return zT

            def mm_node(w_sb, rhs, act_func, act_bias, out_ap):
                # W.T @ rhs over GCOLS in bank-aligned chunks, then one
                # whole-width ACT pass PSUM -> SBUF.
                ps = ps_take()
                for c0, c1 in [(0, 512), (512, 1024), (1024, GCOLS)]:
                    nc.tensor.matmul(ps[:, c0:c1], w_sb[:], rhs[:, c0:c1],
                                     start=True, stop=True)
                nc.scalar.activation(out=out_ap, in_=ps[:, 0:GCOLS],
                                     func=act_func, bias=act_bias, scale=1.0)

            def node_phase(g, zT=None):
                if zT is None:
                    zT = z_load(g)
                # edge layer-0 node terms: ea = exp(a+ib0), eb = exp(b) (x2)
                ea = nodes.tile([H, B, GT], F16)
                eb_ext = nodes.tile([H, 2 * B, GT], F16)
                ea_f = ea[:].rearrange("p r t -> p (r t)")
                eb_f = eb_ext[:].rearrange("p r t -> p (r t)")
                mm_node(Wa_sb, zT, AF.Exp, ib0_c[:], ea_f[:, 0:GCOLS])
                mm_node(Wb_sb, zT, AF.Exp, 0.0, eb_f[:, 0:GCOLS])
                nc.vector.tensor_copy(eb_f[:, GCOLS:2 * GCOLS], eb_f[:, 0:GCOLS])

                # self-dynamics MLP (softplus = exp pass + ln pass)
                u0 = nodes.tile([H, GCOLS], F32, tag="u0")
                mm_node(fW0_sb, zT, AF.Exp, fb0_c[:], u0[:])
                h0s = nodes.tile([H, GCOLS], F16, tag="h0s")
                nc.scalar.activation(out=h0s[:], in_=u0[:], func=AF.Ln,
                                     bias=1.0, scale=1.0)
                u1 = nodes.tile([H, GCOLS], F32, tag="u1")
                mm_node(fW1_sb, h0s, AF.Exp, fb1_c[:], u1[:])
                h1s = nodes.tile([H, B, GT], F16, tag="h1s")
                nc.scalar.activation(
                    out=h1s[:].rearrange("p r t -> p (r t)"), in_=u1[:],
                    func=AF.Ln, bias=1.0, scale=1.0)
                return ea, eb_ext, h1s

            def edge_phase(g, ea, eb_ext, h1s, prefetch_g=None):
                nxt = None
                out_sb = outs.tile([NF, B, GT], F32)

                # grid combine for the whole group: t0(d,r,t) = ea_r*eb_{r+d}
                t0g = grids.tile([H, B - 1, B, GT], F16, tag="t0g")
                for d in range(1, B):
                    nc.vector.tensor_mul(
                        t0g[:, d - 1, :, :], ea[:], eb_ext[:, d:d + B, :])

                def grid_ln(k):
                    tsl = slice(k * TT, (k + 1) * TT)
                    g0 = grids.tile([H, GRID], F16, tag="g0")
                    nc.scalar.activation(
                        out=g0[:], in_=t0g[:, :, :, tsl], func=AF.Ln,
                        bias=1.0, scale=1.0)
                    return g0

                g0 = grid_ln(0)
                for k in range(NBLK):
                    if prefetch_g is not None and k == 1:
                        nxt = node_phase(prefetch_g)
                    tsl = slice(k * TT, (k + 1) * TT)
                    g0_next = grid_ln(k + 1) if k + 1 < NBLK else None

                    # layer 1: u = exp(iW1.T g0 + ib1) in two halves
                    u_e = grids.tile([H, GRID], F32, tag="u_e")
                    for base, width in [(0, 1536), (1536, GRID - 1536)]:
                        ps = ps_take()
                        for c0 in range(0, width, 512):
                            c1 = min(width, c0 + 512)
                            nc.tensor.matmul(
                                ps[:, c0:c1], iW1_sb[:],
                                g0[:, base + c0:base + c1],
                                start=True, stop=True)
                        nc.scalar.activation(
                            out=u_e[:, base:base + width], in_=ps[:, 0:width],
                            func=AF.Exp, bias=ib1_c[:], scale=1.0)
                    h1e = grids.tile([H, B - 1, B, TT], F16, tag="h1e")
                    nc.scalar.activation(
                        out=h1e[:].rearrange("p d r t -> p (d r t)"),
                        in_=u_e[:], func=AF.Ln, bias=1.0, scale=1.0)

                    # sum over the 9 senders on DVE (fp16 2x) so dz is two
                    # matmuls instead of ten (each costs ~165ns fixed).
                    s4 = grids.tile([H, 4, B, TT], F16, tag="s4")
                    nc.vector.tensor_add(s4[:], h1e[:, 0:4, :, :],
                                         h1e[:, 4:8, :, :])
                    s2 = grids.tile([H, 2, B, TT], F16, tag="s2")
                    nc.vector.tensor_add(s2[:], s4[:, 0:2, :, :],
                                         s4[:, 2:4, :, :])
                    s3 = grids.tile([H, B, TT], F16, tag="s3")
                    nc.vector.tensor_add(s3[:], s2[:, 0, :, :],
                                         s2[:, 1, :, :])
                    hsum = grids.tile([H, B, TT], F16, tag="hsum")
                    nc.vector.tensor_add(hsum[:], s3[:], h1e[:, 8, :, :])

                    # dz = fW2.T h1s + iW2.T hsum
                    dzp = dz_ps.tile([NF, B * TT], F32)
                    nc.tensor.matmul(dzp[:], fW2_sb[:],
                                     h1s[:, :, tsl], start=True, stop=False)
                    nc.tensor.matmul(dzp[:], iW2_sb[:], hsum[:],
                                     start=False, stop=True)
                    nc.vector.tensor_scalar_add(
                        out_sb[:, :, tsl], dzp[:], bias2_c[:])
                    g0 = g0_next

                # transpose back and store (contiguous)
                ot_ps = ps_take()
                out_f = out_sb[:].rearrange("p r t -> p (r t)")
                for r in range(B):
                    nc.tensor.transpose(
                        ot_ps[:, r * NF:(r + 1) * NF],
                        out_f[:, r * 128:(r + 1) * 128], ident4[:])
                outT = zio.tile([128, B, NF], F32, tag="outT")
                nc.vector.tensor_copy(outT[:], ot_ps[:, 0:B * NF])
                nc.sync.dma_start(out=out_v[g],
                                  in_=outT[:].rearrange("p c f -> p (c f)"))
                return nxt

            zT0 = z_load(0)

            def weight_tile(p, fdim, name, src_ap):
                stage = const.tile([p, fdim], F32, tag=f"wstage_{name}")
                nc.sync.dma_start(out=stage[:], in_=src_ap)
                t = const.tile([p, fdim], F16, tag=f"w_{name}")
                nc.vector.tensor_copy(t[:], stage[:])
                return t

            fW0_sb = weight_tile(NF, H, "fW0", w["fW0"][:])
            fW1_sb = weight_tile(H, H, "fW1", w["fW1"][:])
            fW2_sb = weight_tile(H, NF, "fW2", w["fW2"][:])
            iW1_sb = weight_tile(H, H, "iW1", w["iW1"][:])
            iW2_sb = weight_tile(H, NF, "iW2", w["iW2"][:])
            Wa_sb = weight_tile(NF, H, "Wa", w["iW0"][0:NF, :])
            # Wb = [-iW0[0:2]; iW0[4:6]] prepared host-side (param "Wb")
            Wb_sb = weight_tile(NF, H, "Wb", w["Wb"][:])

            def bias_col(p, name):
                t = const.tile([p, 1], F32, tag=f"bias_{name}")
                nc.sync.dma_start(out=t[:],
                                  in_=w[name].rearrange("(a b) -> a b", b=1))
                return t

            fb0_c = bias_col(H, "fb0")
            fb1_c = bias_col(H, "fb1")
            ib0_c = bias_col(H, "ib0")
            ib1_c = bias_col(H, "ib1")
            # bias2 = fb2 + 9*ib2 prepared host-side (param "bias2")
            bias2_c = bias_col(NF, "bias2")

            # software-pipelined: group g+1's node phase is emitted after
            # group g's first edge block so ACT never starves.
            tiles = node_phase(0, zT0)
            for g in range(ngroup):
                pf = g + 1 if g + 1 < ngroup else None
                tiles = edge_phase(g, *tiles, prefetch_g=pf)

    nc.finalize()
    return nc


_NC_CACHE = {}


def _get_nc():
    if "nc" not in _NC_CACHE:
        _NC_CACHE["nc"] = build()
    return _NC_CACHE["nc"]


def run(inputs, trace=False, **kwargs):
    """Shard, run on 8 cores, gather. Returns (out, BassKernelResults)."""
    nc = _get_nc()
    z = np.ascontiguousarray(np.asarray(inputs["z"], dtype=np.float32))
    assert z.shape == (N_TRAJ * B, NF), z.shape
    weights = {k: np.ascontiguousarray(np.asarray(inputs[k], dtype=np.float32))
               for k in WEIGHT_NAMES}
    iW0 = weights["iW0"]
    weights["Wb"] = np.ascontiguousarray(
        np.concatenate([-iW0[0:NDIM], iW0[2 * NDIM:3 * NDIM]], axis=0))
    weights["bias2"] = np.ascontiguousarray(
        weights["fb2"] + (B - 1) * weights["ib2"])
    for name in ("fb2", "ib2"):
        del weights[name]
    in_maps = []
    for c in range(N_CORES):
        m = dict(weights)
        m["z"] = z[c * ROWS:(c + 1) * ROWS]
        in_maps.append(m)
    res = run_bass_kernel_spmd(nc, in_maps, list(range(N_CORES)),
                               trace=trace, **kwargs)
    out = np.concatenate([res.results[c]["out"] for c in range(N_CORES)],
                         axis=0)
    return out, res


def kernel(**inputs) -> np.ndarray:
    out, _ = run(inputs)
    return out


# revision 8
# speedup vs baseline: 1.2741x; 1.0974x over previous
"""Trainium2 Bass kernel for nn_BaseIODEModel (GNN message-passing ODE field).

Data-parallel over trajectories: z [81920, 4] is split across 8 NeuronCores
along dim 0 (1024 trajectories / 10240 rows per core); the small MLP weights
are replicated. Edge gather/softplus/sum is local per trajectory, so there is
no cross-device communication.

Per-core program, fp16 matmul datapath (PE runs 2-byte dtypes at 1 col/cycle
vs 2-4 cycles for fp32r; fp32 PSUM accumulation throughout; ranges verified:
max |a_r + b_s| < 9.2 so exp products stay < 1e4, well inside fp16):

  zT = transpose(z)                          [4, 1280] per group (PE)
  interaction layer 0 factorizes over edges:
       pre(r,s) = a_r + b_s + ib0,  a = [iW0_p; iW0_vr].T z,
                                    b = [-iW0_p; iW0_vs].T z
       ea = exp(a + ib0), eb = exp(b)   (ACT, node columns only)
       t0(d,r,t) = ea_r * eb_{(r+d)%B}  (DVE fp16 2x, one op per shift d)
       g0 = ln(1 + t0)                  (ACT, one 2880-col op per block)
  layer 1:  u = exp(iW1.T g0 + ib1)     (ACT, PSUM in, f32 out)
            h1e = ln(1 + u)             (ACT, one 2880-col op per block)
  dz_int = sum_d iW2.T h1e[:, d]        (PE, PSUM accumulation)
  self-dynamics MLP: same exp/ln softplus pairs on node columns.
  out = dz_self + dz_int + (fb2 + 9*ib2), PE-transposed back to row-major.

ACT (the only transcendental engine; throughput is dtype-independent at
~0.86 ns/col) is the roofline: 3 edge passes x 92160 cols + 6 node passes
x 10240 cols per core. All ACT instructions are sized 1280-2880 cols to
amortize the ~200 ns per-instruction access latency.
"""

import numpy as np

import concourse.bass as bass
import concourse.hw_specs as _hw_specs
import concourse.mybir as _mybir_for_tables
from concourse import bacc


def _patch_activation_tables():
    """Make Exp and Ln resolve to the combined natural_log_exp_and_others
    ACT table set. Bacc's insert_act_table_loads picks the first set that
    contains each function, which puts Exp and Ln in two different sets and
    inserts a ~1.3us ACT_TABLE_LOAD at every exp<->ln alternation. Filtering
    the other sets' exp/ln entries keeps set ids stable (index into
    act_info.json) while forcing the shared set."""
    if getattr(_hw_specs, "_nle_patched", False):
        return
    orig = _hw_specs.get_activation_tables
    comb = "natural_log_exp_and_others"
    EXP = _mybir_for_tables.ActivationFunctionType.Exp
    LN = _mybir_for_tables.ActivationFunctionType.Ln

    def patched(module_arch):
        tables = orig(module_arch)
        if comb in tables and EXP in tables[comb] and LN in tables[comb]:
            for name, funcs in tables.items():
                if name != comb:
                    funcs.discard(EXP)
                    funcs.discard(LN)
        return tables

    _hw_specs.get_activation_tables = patched
    _hw_specs._nle_patched = True
    import concourse.bacc as _bacc_mod
    if getattr(_bacc_mod, "get_activation_tables", None) is orig:
        _bacc_mod.get_activation_tables = patched


_patch_activation_tables()
import concourse.mybir as mybir
import concourse.tile as tile
from concourse.bass_utils import run_bass_kernel_spmd
from concourse.masks import make_identity

F32 = mybir.dt.float32
F16 = mybir.dt.float16
AF = mybir.ActivationFunctionType

B = 10          # objects per trajectory
NDIM = 2
NF = 2 * NDIM   # 4 features per node
H = 128         # hidden width (both MLPs)

N_CORES = 8
N_TRAJ = 8192             # total trajectories
N_LOC = N_TRAJ // N_CORES  # 1024 trajectories per core
ROWS = N_LOC * B          # 10240 node rows per core
GT = 128                  # trajectories per group
NGROUP = N_LOC // GT      # 8 groups
GCOLS = GT * B            # 1280 node cols per group
TT = 32                   # trajectories per edge block
NBLK = GT // TT           # 4 edge blocks per group
GRID = TT * (B - 1) * B   # 2880 grid cols per block

WEIGHT_NAMES = [
    "fW0", "fb0", "fW1", "fb1", "fW2", "fb2",
    "iW0", "ib0", "iW1", "ib1", "iW2", "ib2",
]


def build(ngroup=NGROUP):
    nc = bacc.Bacc()
    rows = ngroup * GCOLS

    z = nc.declare_dram_parameter("z", [rows, NF], F32, isOutput=False)
    w = {}
    for name, shp in [
        ("fW0", [NF, H]), ("fb0", [H]), ("fW1", [H, H]), ("fb1", [H]),
        ("fW2", [H, NF]),
        ("iW0", [3 * NDIM, H]), ("ib0", [H]), ("iW1", [H, H]), ("ib1", [H]),
        ("iW2", [H, NF]),
        ("Wb", [NF, H]), ("bias2", [NF]),
    ]:
        w[name] = nc.declare_dram_parameter(name, shp, F32, isOutput=False)
    out = nc.declare_dram_parameter("out", [rows, NF], F32, isOutput=True)

    # DRAM views: rows=(g,p,c): partition p = trajectory, c = node.
    # Per-partition runs are 10*4 contiguous f32 (160B DMA bursts).
    z_v = z.rearrange("(g p c) f -> g p (c f)", g=ngroup, p=128, c=B)
    out_v = out.rearrange("(g p c) f -> g p (c f)", g=ngroup, p=128, c=B)

    with tile.TileContext(nc) as tc:
        with (
            tc.tile_pool(name="const", bufs=1) as const,
            tc.tile_pool(name="zio", bufs=2) as zio,
            tc.tile_pool(name="nodes", bufs=2) as nodes,
            tc.tile_pool(name="grids", bufs=2) as grids,
            tc.tile_pool(name="outs", bufs=2) as outs,
            # One shared PSUM ring for all [128, x] matmul outputs:
            # 1536 f32 = 3 banks; x2 bufs = 6 banks.  dz: [4,320] x2 = 2.
            tc.tile_pool(name="ab_ps", bufs=2, space="PSUM") as ab_ps,
            tc.tile_pool(name="dz_ps", bufs=2, space="PSUM") as dz_ps,
        ):
            ident128 = const.tile([128, 128], F16)
            make_identity(nc, ident128)
            ident4 = const.tile([NF, NF], F32)
            make_identity(nc, ident4)

            def ps_take():
                ab_take = ab_ps.tile([128, 1536], F32, tag="ab")
                return ab_take

            def z_load(g):
                # load z (contiguous), cast to fp16, transpose to [4, 1280]
                z_sb = zio.tile([128, B, NF], F32)  # [traj, node, feat]
                nc.sync.dma_start(out=z_sb[:].rearrange("p c f -> p (c f)"),
                                  in_=z_v[g])
                zb = zio.tile([128, B, NF], F16)
                nc.vector.tensor_copy(zb[:], z_sb[:])
                zt_ps = ps_take()
                ztv = zt_ps[:].bitcast(F16)  # [128, 3072] fp16 view
                for r in range(B):
                    nc.tensor.transpose(
                        ztv[0:NF, r * 128:(r + 1) * 128], zb[:, r, :],
                        ident128[:])
                zT = nodes.tile([NF, GCOLS], F16)
                nc.vector.tensor_copy(zT[:], ztv[0:NF, 0:GCOLS])
                return zT

            def mm_node(w_sb, rhs, act_func, act_bias, out_ap):
                # W.T @ rhs over GCOLS in bank-aligned chunks, then one
                # whole-width ACT pass PSUM -> SBUF.
                ps = ps_take()
                for c0, c1 in [(0, 512), (512, 1024), (1024, GCOLS)]:
                    nc.tensor.matmul(ps[:, c0:c1], w_sb[:], rhs[:, c0:c1],
                                     start=True, stop=True)
                nc.scalar.activation(out=out_ap, in_=ps[:, 0:GCOLS],
                                     func=act_func, bias=act_bias, scale=1.0)

            def node_phase(g, zT=None):
                if zT is None:
                    zT = z_load(g)
                # edge layer-0 node terms: ea = exp(a+ib0), eb = exp(b) (x2)
                ea = nodes.tile([H, B, GT], F16)
                eb_ext = nodes.tile([H, 2 * B, GT], F16)
                ea_f = ea[:].rearrange("p r t -> p (r t)")
                eb_f = eb_ext[:].rearrange("p r t -> p (r t)")
                mm_node(Wa_sb, zT, AF.Exp, ib0_c[:], ea_f[:, 0:GCOLS])
                mm_node(Wb_sb, zT, AF.Exp, 0.0, eb_f[:, 0:GCOLS])
                nc.vector.tensor_copy(eb_f[:, GCOLS:2 * GCOLS], eb_f[:, 0:GCOLS])

                # self-dynamics MLP (softplus = exp pass + ln pass)
                u0 = nodes.tile([H, GCOLS], F32, tag="u0")
                mm_node(fW0_sb, zT, AF.Exp, fb0_c[:], u0[:])
                h0s = nodes.tile([H, GCOLS], F16, tag="h0s")
                nc.scalar.activation(out=h0s[:], in_=u0[:], func=AF.Ln,
                                     bias=1.0, scale=1.0)
                u1 = nodes.tile([H, GCOLS], F32, tag="u1")
                mm_node(fW1_sb, h0s, AF.Exp, fb1_c[:], u1[:])
                h1s = nodes.tile([H, B, GT], F16, tag="h1s")
                nc.scalar.activation(
                    out=h1s[:].rearrange("p r t -> p (r t)"), in_=u1[:],
                    func=AF.Ln, bias=1.0, scale=1.0)

                # grid combine for the whole group: t0(d,r,t) = ea_r*eb_{r+d}
                # (emitted here so the next group's first grid ln is never
                # gated on late DVE work)
                t0g = grids.tile([H, B - 1, B, GT], F16, tag="t0g")
                for d in range(1, B):
                    nc.vector.tensor_mul(
                        t0g[:, d - 1, :, :], ea[:], eb_ext[:, d:d + B, :])
                return t0g, h1s

            def edge_phase(g, t0g, h1s, prefetch_g=None):
                nxt = None
                out_sb = outs.tile([NF, B, GT], F32)

                def grid_ln(k):
                    tsl = slice(k * TT, (k + 1) * TT)
                    g0 = grids.tile([H, GRID], F16, tag="g0", bufs=3)
                    nc.scalar.activation(
                        out=g0[:], in_=t0g[:, :, :, tsl], func=AF.Ln,
                        bias=1.0, scale=1.0)
                    return g0

                # two-block-deep ln lookahead keeps ACT fed while the PE
                # works on the L1 matmuls of the current block.
                pend = [grid_ln(0), grid_ln(1)]
                for k in range(NBLK):
                    if prefetch_g is not None and k == 1:
                        nxt = node_phase(prefetch_g)
                    tsl = slice(k * TT, (k + 1) * TT)
                    g0 = pend.pop(0)
                    if k + 2 < NBLK:
                        pend.append(grid_ln(k + 2))

                    # layer 1: u = exp(iW1.T g0 + ib1) in two halves
                    u_e = grids.tile([H, GRID], F32, tag="u_e")
                    for base, width in [(0, 1536), (1536, GRID - 1536)]:
                        ps = ps_take()
                        for c0 in range(0, width, 512):
                            c1 = min(width, c0 + 512)
                            nc.tensor.matmul(
                                ps[:, c0:c1], iW1_sb[:],
                                g0[:, base + c0:base + c1],
                                start=True, stop=True)
                        nc.scalar.activation(
                            out=u_e[:, base:base + width], in_=ps[:, 0:width],
                            func=AF.Exp, bias=ib1_c[:], scale=1.0)
                    h1e = grids.tile([H, B - 1, B, TT], F16, tag="h1e")
                    nc.scalar.activation(
                        out=h1e[:].rearrange("p d r t -> p (d r t)"),
                        in_=u_e[:], func=AF.Ln, bias=1.0, scale=1.0)

                    # sum over the 9 senders on DVE (fp16 2x) so dz is two
                    # matmuls instead of ten (each costs ~165ns fixed).
                    s4 = grids.tile([H, 4, B, TT], F16, tag="s4")
                    nc.vector.tensor_add(s4[:], h1e[:, 0:4, :, :],
                                         h1e[:, 4:8, :, :])
                    s2 = grids.tile([H, 2, B, TT], F16, tag="s2")
                    nc.vector.tensor_add(s2[:], s4[:, 0:2, :, :],
                                         s4[:, 2:4, :, :])
                    s3 = grids.tile([H, B, TT], F16, tag="s3")
                    nc.vector.tensor_add(s3[:], s2[:, 0, :, :],
                                         s2[:, 1, :, :])
                    hsum = grids.tile([H, B, TT], F16, tag="hsum")
                    nc.vector.tensor_add(hsum[:], s3[:], h1e[:, 8, :, :])

                    # dz = fW2.T h1s + iW2.T hsum
                    dzp = dz_ps.tile([NF, B * TT], F32)
                    nc.tensor.matmul(dzp[:], fW2_sb[:],
                                     h1s[:, :, tsl], start=True, stop=False)
                    nc.tensor.matmul(dzp[:], iW2_sb[:], hsum[:],
                                     start=False, stop=True)
                    nc.vector.tensor_scalar_add(
                        out_sb[:, :, tsl], dzp[:], bias2_c[:])

                # transpose back and store (contiguous)
                ot_ps = ps_take()
                out_f = out_sb[:].rearrange("p r t -> p (r t)")
                for r in range(B):
                    nc.tensor.transpose(
                        ot_ps[:, r * NF:(r + 1) * NF],
                        out_f[:, r * 128:(r + 1) * 128], ident4[:])
                outT = zio.tile([128, B, NF], F32, tag="outT")
                nc.vector.tensor_copy(outT[:], ot_ps[:, 0:B * NF])
                nc.sync.dma_start(out=out_v[g],
                                  in_=outT[:].rearrange("p c f -> p (c f)"))
                return nxt

            zT0 = z_load(0)

            def weight_tile(p, fdim, name, src_ap):
                stage = const.tile([p, fdim], F32, tag=f"wstage_{name}")
                nc.sync.dma_start(out=stage[:], in_=src_ap)
                t = const.tile([p, fdim], F16, tag=f"w_{name}")
                nc.vector.tensor_copy(t[:], stage[:])
                return t

            fW0_sb = weight_tile(NF, H, "fW0", w["fW0"][:])
            fW1_sb = weight_tile(H, H, "fW1", w["fW1"][:])
            fW2_sb = weight_tile(H, NF, "fW2", w["fW2"][:])
            iW1_sb = weight_tile(H, H, "iW1", w["iW1"][:])
            iW2_sb = weight_tile(H, NF, "iW2", w["iW2"][:])
            Wa_sb = weight_tile(NF, H, "Wa", w["iW0"][0:NF, :])
            # Wb = [-iW0[0:2]; iW0[4:6]] prepared host-side (param "Wb")
            Wb_sb = weight_tile(NF, H, "Wb", w["Wb"][:])

            def bias_col(p, name):
                t = const.tile([p, 1], F32, tag=f"bias_{name}")
                nc.sync.dma_start(out=t[:],
                                  in_=w[name].rearrange("(a b) -> a b", b=1))
                return t

            fb0_c = bias_col(H, "fb0")
            fb1_c = bias_col(H, "fb1")
            ib0_c = bias_col(H, "ib0")
            ib1_c = bias_col(H, "ib1")
            # bias2 = fb2 + 9*ib2 prepared host-side (param "bias2")
            bias2_c = bias_col(NF, "bias2")

            # software-pipelined: group g+1's node phase is emitted after
            # group g's first edge block so ACT never starves.
            tiles = node_phase(0, zT0)
            for g in range(ngroup):
                pf = g + 1 if g + 1 < ngroup else None
                tiles = edge_phase(g, *tiles, prefetch_g=pf)

    nc.finalize()
    return nc


_NC_CACHE = {}


def _get_nc():
    if "nc" not in _NC_CACHE:
        _NC_CACHE["nc"] = build()
    return _NC_CACHE["nc"]


def run(inputs, trace=False, **kwargs):
    """Shard, run on 8 cores, gather. Returns (out, BassKernelResults)."""
    nc = _get_nc()
    z = np.ascontiguousarray(np.asarray(inputs["z"], dtype=np.float32))
    assert z.shape == (N_TRAJ * B, NF), z.shape
    weights = {k: np.ascontiguousarray(np.asarray(inputs[k], dtype=np.float32))
               for k in WEIGHT_NAMES}
    iW0 = weights["iW0"]
    weights["Wb"] = np.ascontiguousarray(
        np.concatenate([-iW0[0:NDIM], iW0[2 * NDIM:3 * NDIM]], axis=0))
    weights["bias2"] = np.ascontiguousarray(
        weights["fb2"] + (B - 1) * weights["ib2"])
    for name in ("fb2", "ib2"):
        del weights[name]
    in_maps = []
    for c in range(N_CORES):
        m = dict(weights)
        m["z"] = z[c * ROWS:(c + 1) * ROWS]
        in_maps.append(m)
    res = run_bass_kernel_spmd(nc, in_maps, list(range(N_CORES)),
                               trace=trace, **kwargs)
    out = np.concatenate([res.results[c]["out"] for c in range(N_CORES)],
                         axis=0)
    return out, res


def kernel(**inputs) -> np.ndarray:
    out, _ = run(inputs)
    return out
